# revision 7
# baseline (speedup 1.0000x reference)
"""nn_BlockMoVaE Trainium2 kernel: attention + MoVaE block on 8 NeuronCores.

Sharding:
  - Attention: head-parallel within each batch. Core c handles batch b=c//4,
    head group g=c%4 (4 q-heads, 2 kv-heads), over the full 2048-token batch.
    wo contributions are ReduceScattered over the 4-core batch group so core g
    receives the summed attention output for its own token slice
    [512g, 512(g+1)).
  - MoVaE: token-parallel. Each core runs the router + all 8 MLP experts +
    both VE table gathers for its own 512 tokens. No collective needed.

Precision: fp32r (full-rate PE) for bulk matmuls; exact fp32 for the
router path (MoE-input RMS statistics + router logits) because top-2 expert
selection has ~1e-5 margins.
"""
import sys
sys.path.insert(0, '/opt/trn_rl_repo')

from contextlib import ExitStack
import numpy as np

import concourse.bass as bass
import concourse.bacc as bacc
import concourse.tile as tile
from concourse import mybir
from concourse import bass_isa
from concourse.masks import make_identity
from concourse.bass_utils import run_bass_kernel_spmd

F32 = mybir.dt.float32
F32R = mybir.dt.float32r
AF = mybir.ActivationFunctionType
OP = mybir.AluOpType

B, T, C = 2, 2048, 1024
NH, NKV, HD = 16, 8, 64
E_MLP, E_VE, HID, VOCAB = 8, 2, 1024, 32000
E_TOT = E_MLP + E_VE
EPS = 1e-6
NCORES = 8
CT = C // 128            # 8 c-tiles
HPC = 4                  # heads per core
KVPC = 2                 # kv heads per core
NTOK = 512               # tokens owned per core
NBLK = T // 512          # 4 n-blocks of 512
SQ = 4                   # q-tiles of 512 per core
WKG = 3                  # k-groups per exp wave


def _mask_plan(w):
    """plan[s] = [(kg_list, mask_slice_offset_or_None), ...] per exp wave.
    mask_stack: (NS, 128, 512) fp32 0/1 visibility slices."""
    plan = []
    slices = []
    for s in range(SQ):
        kgs = list(range(4 * (s + 1)))
        waves = [kgs[i:i + WKG] for i in range(0, len(kgs), WKG)]
        wplan = []
        for wkgs in waves:
            arrs = []
            need = False
            for kg in wkgs:
                p = np.arange(128)[:, None] + kg * 128      # key position
                f = np.arange(512)[None, :] + 512 * s       # query position
                m = ((p <= f) & ((f - p) <= w)).astype(np.float32)
                arrs.append(m)
                if (m != 1.0).any():
                    need = True
            if need:
                off = len(slices)
                slices.extend(arrs)
                wplan.append((wkgs, off))
            else:
                wplan.append((wkgs, None))
        plan.append(wplan)
    if not slices:
        slices.append(np.ones((128, 512), np.float32))
    return plan, np.stack(slices, axis=0)


def _build(window_size, n_mask_slices):
    nc = bacc.Bacc(None, target_bir_lowering=False, debug=False, num_devices=NCORES)
    plan, _ = _mask_plan(window_size)

    # ---- parameters ----
    P = nc.declare_dram_parameter
    xT = P("xT", [C, T], F32, isOutput=False)
    own_xT = P("own_xT", [C, NTOK], F32, isOutput=False)
    cosT = P("cosT", [32, T], F32, isOutput=False)
    sinT = P("sinT", [64, T], F32, isOutput=False)
    wqT = P("wqT", [C, HPC * HD], F32R, isOutput=False)
    wkT = P("wkT", [C, KVPC * HD], F32R, isOutput=False)
    wvT = P("wvT", [C, KVPC * HD], F32R, isOutput=False)
    woT = P("woT", [HPC * HD, C], F32R, isOutput=False)
    routerT = P("routerT", [C, E_TOT], F32, isOutput=False)
    fcT = P("fcT", [E_MLP, C, HID], F32R, isOutput=False)
    projT = P("projT", [E_MLP, HID, C], F32R, isOutput=False)
    ve = P("ve", [E_VE, VOCAB, C], F32, isOutput=False)
    tok_idx = P("tok_idx", [128, NTOK // 16], mybir.dt.int16, isOutput=False)
    masks = P("masks", [n_mask_slices, 128, 512], F32R, isOutput=False)
    ind_a = P("ind_a", [128, 2], F32, isOutput=False)
    ind_b = P("ind_b", [2, 128], F32, isOutput=False)
    outT = P("outT", [C, NTOK], F32, isOutput=True)
    routT = P("routT", [E_TOT, NTOK], F32, isOutput=True)

    xT3 = xT.rearrange("(po pi) n -> pi po n", pi=128)
    own_xT3 = own_xT.rearrange("(po pi) n -> pi po n", pi=128)
    wqT3 = wqT.rearrange("(po pi) m -> pi po m", pi=128)
    wkT3 = wkT.rearrange("(po pi) m -> pi po m", pi=128)
    wvT3 = wvT.rearrange("(po pi) m -> pi po m", pi=128)
    woT3 = woT.rearrange("(po pi) m -> pi po m", pi=128)
    routerT3 = routerT.rearrange("(po pi) m -> pi po m", pi=128)

    with tile.TileContext(nc) as tc, ExitStack() as top:
        const = top.enter_context(tc.tile_pool(name="const", bufs=1))
        dram = top.enter_context(tc.tile_pool(name="dram", bufs=1, space="DRAM"))

        ident = const.tile([128, 128], F32)
        make_identity(nc, ident)
        ones_col = const.tile([128, 1], F32)
        nc.vector.memset(ones_col[:], 1.0)
        ones_row = const.tile([1, 128], F32)
        nc.vector.memset(ones_row[:], 1.0)
        eps_b = const.tile([128, 1], F32)
        nc.vector.memset(eps_b[:], EPS)
        neg8_b = const.tile([128, 1], F32)
        nc.vector.memset(neg8_b[:], -8.0)
        ia = const.tile([128, 2], F32)
        ib = const.tile([2, 128], F32)
        nc.sync.dma_start(ia[:], ind_a[:, :])
        nc.sync.dma_start(ib[:], ind_b[:, :])

        attn_part = dram.tile([4, C, NTOK], F32)
        attn_own = dram.tile([C, NTOK], F32)

        # ================= attention =================
        with ExitStack() as attn:
            persist = attn.enter_context(tc.tile_pool(name="persist", bufs=1))
            cos_sb = persist.tile([128, T], F32)
            _cap = cosT[:, :]
            nc.gpsimd.dma_start(cos_sb[:], bass.AP(
                tensor=_cap.tensor, offset=0, ap=[[0, 4], [T, 32], [1, T]]))
            sin_sb = persist.tile([128, T], F32)
            _sap = sinT[:, :]
            nc.gpsimd.dma_start(sin_sb[:], bass.AP(
                tensor=_sap.tensor, offset=0, ap=[[0, 2], [T, 64], [1, T]]))

            q_rot = persist.tile([128, 2, T], F32R)
            k_rot = persist.tile([128, T], F32R)
            k_alt = persist.tile([128, T], F32R)
            v_aug = persist.tile([128, T // 128, KVPC, HD + 1], F32R)
            y_sb = persist.tile([128, 2, T], F32R)
            # fp32r memset trips an ISA check; write the ones column via a
            # broadcast fp32 copy instead (DVE rounds to fp32r on store)
            nc.vector.tensor_copy(v_aug[:, :, :, HD:HD + 1],
                                  ones_col[:].to_broadcast([128, T // 128, KVPC, 1]))

            def rope_qknorm(pool, spool, ps, dst, col0, ncols):
                # tsw[r] = ps[r^32] * sin_sb[r]  (psum input: cross-base legal)
                tsw = pool.tile([128, 512], F32, tag="rope_tsw")
                tcos = pool.tile([128, 512], F32, tag="rope_tcos")
                for q0_ in (0, 32, 64, 96):
                    pq = q0_ ^ 32
                    nc.vector.tensor_mul(tsw[q0_:q0_ + 32, :ncols],
                                         ps[pq:pq + 32, :],
                                         sin_sb[q0_:q0_ + 32, col0:col0 + ncols])
                nc.vector.tensor_mul(tcos[:, :ncols], ps, cos_sb[:, col0:col0 + ncols])
                rot = pool.tile([128, 512], F32, tag="rope_rot")
                nc.vector.tensor_add(rot[:, :ncols], tcos[:, :ncols], tsw[:, :ncols])
                sq = pool.tile([128, 512], F32, tag="rope_sq")
                nc.vector.tensor_mul(sq[:, :ncols], rot[:, :ncols], rot[:, :ncols])
                ssq = spool.tile([2, 512], F32, tag="rope_ssq")
                nc.tensor.matmul(ssq[:, :ncols], ia[:], sq[:, :ncols])
                rstd = pool.tile([2, 512], F32, tag="rope_rstd")
                nc.scalar.activation(rstd[:, :ncols], ssq[:, :ncols], AF.Sqrt,
                                     bias=eps_b[0:2], scale=1.0 / HD)
                nc.vector.reciprocal(rstd[:, :ncols], rstd[:, :ncols])
                bc = spool.tile([128, 512], F32, tag="rope_bc")
                nc.tensor.matmul(bc[:, :ncols], ib[:], rstd[:, :ncols])
                nc.vector.tensor_mul(dst, rot[:, :ncols], bc[:, :ncols])

            # ---- stage 1: norm + projections ----
            with ExitStack() as s1:
                pool = s1.enter_context(tc.tile_pool(name="s1", bufs=2))
                ppool = s1.enter_context(tc.tile_pool(name="s1p", bufs=2, space="PSUM"))
                spool = s1.enter_context(tc.tile_pool(name="s1s", bufs=1, space="PSUM"))
                wq_sb = pool.tile([128, CT, HPC * HD], F32R, tag="wq")
                wk_sb = pool.tile([128, CT, KVPC * HD], F32R, tag="wk")
                wv_sb = pool.tile([128, CT, KVPC * HD], F32R, tag="wv")
                nc.sync.dma_start(wq_sb[:], wqT3)
                nc.sync.dma_start(wk_sb[:], wkT3)
                nc.sync.dma_start(wv_sb[:], wvT3)

                for blk in range(NBLK):
                    c0 = blk * 512
                    xb = pool.tile([128, CT, 512], F32, tag="xb")
                    nc.sync.dma_start(xb[:], xT3[:, :, c0:c0 + 512])
                    acc = pool.tile([128, 512], F32, tag="rmsacc")
                    sqt = pool.tile([128, 512], F32, tag="rmssq")
                    nc.vector.tensor_mul(acc[:], xb[:, 0, :], xb[:, 0, :])
                    for ct in range(1, CT):
                        nc.vector.tensor_mul(sqt[:], xb[:, ct, :], xb[:, ct, :])
                        nc.vector.tensor_add(acc[:], acc[:], sqt[:])
                    ssq = spool.tile([1, 512], F32, tag="rms_ssq")
                    nc.tensor.matmul(ssq[:], ones_col[:], acc[:])
                    gfac = pool.tile([1, 512], F32, tag="rms_g")
                    nc.scalar.activation(gfac[:], ssq[:], AF.Sqrt, bias=eps_b[0:1], scale=1.0 / C)
                    nc.vector.reciprocal(gfac[:], gfac[:])
                    gbc = spool.tile([128, 512], F32, tag="rms_bc")
                    nc.tensor.matmul(gbc[:], ones_row[:], gfac[:])
                    hb = pool.tile([128, CT, 512], F32R, tag="hb")
                    for ct in range(CT):
                        nc.vector.tensor_mul(hb[:, ct, :], xb[:, ct, :], gbc[:])

                    for m in range(2):
                        psq = ppool.tile([128, 512], F32, tag="ps_qk")
                        for ct in range(CT):
                            nc.tensor.matmul(psq[:], wq_sb[:, ct, m * 128:(m + 1) * 128],
                                             hb[:, ct, :], start=(ct == 0), stop=(ct == CT - 1))
                        rope_qknorm(pool, spool, psq[:], q_rot[:, m, c0:c0 + 512], c0, 512)
                    psk = ppool.tile([128, 512], F32, tag="ps_qk")
                    for ct in range(CT):
                        nc.tensor.matmul(psk[:], wk_sb[:, ct, :], hb[:, ct, :],
                                         start=(ct == 0), stop=(ct == CT - 1))
                    rope_qknorm(pool, spool, psk[:], k_rot[:, c0:c0 + 512], c0, 512)
                    for sub in range(4):
                        n0 = c0 + sub * 128
                        psv = ppool.tile([128, 128], F32, tag="ps_v")
                        for ct in range(CT):
                            nc.tensor.matmul(psv[:], hb[:, ct, sub * 128:(sub + 1) * 128],
                                             wv_sb[:, ct, :], start=(ct == 0), stop=(ct == CT - 1))
                        nc.vector.tensor_copy(
                            v_aug[:, n0 // 128, :, 0:HD],
                            psv[:].rearrange("p (h d) -> p h d", h=KVPC))

                nc.sync.dma_start(k_alt[0:64, :], k_rot[64:128, :])
                nc.sync.dma_start(k_alt[64:128, :], k_rot[0:64, :])

            # ---- stage 2: scores / exp / av ----
            with ExitStack() as s2:
                pool = s2.enter_context(tc.tile_pool(name="s2", bufs=3))
                mpool = s2.enter_context(tc.tile_pool(name="s2m", bufs=1))
                sc_ps = s2.enter_context(tc.tile_pool(name="s2sc", bufs=2, space="PSUM"))
                av_ps = s2.enter_context(tc.tile_pool(name="s2av", bufs=1, space="PSUM"))
                fin_ps = s2.enter_context(tc.tile_pool(name="s2fin", bufs=1, space="PSUM"))
                mask_sb = mpool.tile([128, n_mask_slices, 512], F32R)
                nc.sync.dma_start(mask_sb[:], masks.rearrange("s p f -> p s f"))

                for s in range(SQ):
                    q0 = s * 512
                    for h in range(HPC):
                        qb = 64 * (h % 2)
                        kv = h // 2
                        if (h % 2) == (kv % 2):
                            ksrc, kb = k_rot, 64 * (kv % 2)
                        else:
                            ksrc, kb = k_alt, 64 * ((kv + 1) % 2)
                        yacc = av_ps.tile([HD + 1, 512], F32, tag="yacc")
                        nwaves = len(plan[s])
                        for wi, (wkgs, moff) in enumerate(plan[s]):
                            nw = len(wkgs)
                            ps = sc_ps.tile([128, WKG, 512], F32, tag="wave")
                            for i, kg in enumerate(wkgs):
                                nc.tensor.matmul(
                                    ps[:, i, :],
                                    ksrc[kb:kb + 64, kg * 128:(kg + 1) * 128],
                                    q_rot[qb:qb + 64, h // 2, q0:q0 + 512])
                            E = pool.tile([128, WKG, 512], F32R, tag="E")
                            nc.scalar.activation(E[:, :nw, :], ps[:, :nw, :], AF.Exp,
                                                 bias=neg8_b[:], scale=0.125)
                            if moff is not None:
                                nc.vector.tensor_mul(E[:, :nw, :], E[:, :nw, :],
                                                     mask_sb[:, moff:moff + nw, :])
                            for i, kg in enumerate(wkgs):
                                nc.tensor.matmul(yacc[:], v_aug[:, kg, kv, :], E[:, i, :],
                                                 start=(wi == 0 and i == 0),
                                                 stop=(wi == nwaves - 1 and i == nw - 1))
                        rd = pool.tile([1, 512], F32, tag="recip_d")
                        nc.vector.reciprocal(rd[:], yacc[HD:HD + 1, :])
                        dbc = fin_ps.tile([64, 512], F32, tag="dbc")
                        nc.tensor.matmul(dbc[:], ones_row[:, 0:64], rd[:])
                        yav = pool.tile([64, 512], F32, tag="yav")
                        nc.vector.tensor_copy(yav[:], yacc[0:HD, :])
                        nc.vector.tensor_mul(y_sb[qb:qb + 64, h // 2, q0:q0 + 512],
                                             yav[:], dbc[:])

            # ---- stage 3: wo + ReduceScatter ----
            with ExitStack() as s3:
                pool = s3.enter_context(tc.tile_pool(name="s3", bufs=3))
                ppool = s3.enter_context(tc.tile_pool(name="s3p", bufs=4, space="PSUM"))
                wo_sb = pool.tile([128, 2, C], F32R, tag="wo")
                nc.sync.dma_start(wo_sb[:], woT3)
                for s in range(SQ):
                    for ct in range(CT):
                        ps = ppool.tile([128, 512], F32, tag="wops")
                        for kt in range(2):
                            nc.tensor.matmul(ps[:], wo_sb[:, kt, ct * 128:(ct + 1) * 128],
                                             y_sb[:, kt, s * 512:(s + 1) * 512],
                                             start=(kt == 0), stop=(kt == 1))
                        ev = pool.tile([128, 512], F32, tag="woev")
                        nc.vector.tensor_copy(ev[:], ps[:])
                        nc.sync.dma_start(attn_part[s, ct * 128:(ct + 1) * 128, :], ev[:])
                nc.gpsimd.collective_compute(
                    "ReduceScatter", OP.add,
                    replica_groups=[[0, 1, 2, 3], [4, 5, 6, 7]],
                    ins=[attn_part[:].opt()],
                    outs=[attn_own[:].opt()],
                )

        # ================= MoVaE =================
        moe = top.enter_context(tc.tile_pool(name="moe", bufs=1))
        x2 = moe.tile([128, CT, NTOK], F32)
        xf = moe.tile([128, CT, NTOK], F32)
        xfr = moe.tile([128, CT, NTOK], F32R)
        moe_acc = moe.tile([128, CT, NTOK], F32)
        routing = moe.tile([E_TOT, NTOK], F32)
        sparseT = moe.tile([E_TOT, 4, 128], F32)
        ve_w = moe.tile([128, 4, 2], F32)

        attn_own3 = attn_own[:].rearrange("(po pi) n -> pi po n", pi=128)
        with ExitStack() as s4:
            pool = s4.enter_context(tc.tile_pool(name="s4", bufs=1))
            ppool = s4.enter_context(tc.tile_pool(name="s4p", bufs=1, space="PSUM"))
            xo = pool.tile([128, CT, NTOK], F32, tag="xo")
            ao = pool.tile([128, CT, NTOK], F32, tag="ao")
            nc.sync.dma_start(xo[:], own_xT3)
            nc.sync.dma_start(ao[:], attn_own3)
            for ct in range(CT):
                nc.vector.tensor_add(x2[:, ct, :], xo[:, ct, :], ao[:, ct, :])
                nc.vector.tensor_copy(moe_acc[:, ct, :], x2[:, ct, :])
            acc = pool.tile([128, NTOK], F32, tag="racc")
            sqt = pool.tile([128, NTOK], F32, tag="rsq")
            nc.vector.tensor_mul(acc[:], x2[:, 0, :], x2[:, 0, :])
            for ct in range(1, CT):
                nc.vector.tensor_mul(sqt[:], x2[:, ct, :], x2[:, ct, :])
                nc.vector.tensor_add(acc[:], acc[:], sqt[:])
            ssq = ppool.tile([1, NTOK], F32, tag="rssq")
            nc.tensor.matmul(ssq[:], ones_col[:], acc[:])
            gfac = pool.tile([1, NTOK], F32, tag="rg")
            nc.scalar.activation(gfac[:], ssq[:], AF.Sqrt, bias=eps_b[0:1], scale=1.0 / C)
            nc.vector.reciprocal(gfac[:], gfac[:])
            gbc = ppool.tile([128, NTOK], F32, tag="rbc")
            nc.tensor.matmul(gbc[:], ones_row[:], gfac[:])
            for ct in range(CT):
                nc.vector.tensor_mul(xf[:, ct, :], x2[:, ct, :], gbc[:])
                nc.vector.tensor_copy(xfr[:, ct, :], xf[:, ct, :])

            rt_sb = pool.tile([128, CT, E_TOT], F32, tag="rt")
            nc.sync.dma_start(rt_sb[:], routerT3)
            lg = ppool.tile([E_TOT, NTOK], F32, tag="lg")
            for ct in range(CT):
                nc.tensor.matmul(lg[:], rt_sb[:, ct, :], xf[:, ct, :],
                                 start=(ct == 0), stop=(ct == CT - 1))
            lgs = pool.tile([E_TOT, NTOK], F32, tag="lgs")
            nc.vector.tensor_copy(lgs[:], lg[:])
            mx = pool.tile([E_TOT, NTOK], F32, tag="rmx")
            nc.gpsimd.partition_all_reduce(mx[:], lgs[:], channels=E_TOT,
                                           reduce_op=bass_isa.ReduceOp.max)
            sh = pool.tile([E_TOT, NTOK], F32, tag="rsh")
            nc.vector.tensor_sub(sh[:], lgs[:], mx[:])
            ex = pool.tile([E_TOT, NTOK], F32, tag="rex")
            nc.scalar.activation(ex[:], sh[:], AF.Exp)
            sm = pool.tile([E_TOT, NTOK], F32, tag="rsm")
            nc.gpsimd.partition_all_reduce(sm[:], ex[:], channels=E_TOT,
                                           reduce_op=bass_isa.ReduceOp.add)
            nc.vector.reciprocal(sm[:], sm[:])
            nc.vector.tensor_mul(routing[:], ex[:], sm[:])
            nc.sync.dma_start(routT[:, :], routing[:])

            for g in range(4):
                rtk_ps = ppool.tile([128, E_TOT], F32, tag="rtkps")
                nc.tensor.transpose(rtk_ps[:], routing[:, g * 128:(g + 1) * 128],
                                    ident[0:E_TOT, 0:E_TOT])
                rtk = pool.tile([128, E_TOT], F32, tag="rtk")
                nc.vector.tensor_copy(rtk[:], rtk_ps[:])
                m8 = pool.tile([128, 8], F32, tag="m8")
                nc.vector.max(m8[:], rtk[:])
                ssum = pool.tile([128, 1], F32, tag="ssum")
                nc.vector.tensor_add(ssum[:], m8[:, 0:1], m8[:, 1:2])
                nc.vector.tensor_scalar_add(ssum[:], ssum[:], 1e-10)
                nc.vector.reciprocal(ssum[:], ssum[:])
                w0 = pool.tile([128, 1], F32, tag="w0")
                w1 = pool.tile([128, 1], F32, tag="w1")
                nc.vector.tensor_mul(w0[:], m8[:, 0:1], ssum[:])
                nc.vector.tensor_mul(w1[:], m8[:, 1:2], ssum[:])
                eq0 = pool.tile([128, E_TOT], F32, tag="eq0")
                nc.vector.tensor_scalar(eq0[:], rtk[:], m8[:, 0:1], None, OP.is_equal)
                eq1 = pool.tile([128, E_TOT], F32, tag="eq1")
                nc.vector.tensor_scalar(eq1[:], rtk[:], m8[:, 1:2], None, OP.is_equal)
                sp = pool.tile([128, E_TOT], F32, tag="sp")
                nc.vector.tensor_scalar_mul(sp[:], eq0[:], w0[:])
                nc.vector.scalar_tensor_tensor(sp[:], eq1[:], w1[:], sp[:],
                                               op0=OP.mult, op1=OP.add)
                nc.vector.tensor_copy(ve_w[:, g, :], sp[:, E_MLP:])
                spT = ppool.tile([E_TOT, 128], F32, tag="spT")
                nc.tensor.transpose(spT[:], sp[:], ident[:])
                nc.vector.tensor_copy(sparseT[:, g, :], spT[:])

        # ---- dense MoE experts ----
        fcT4 = fcT.rearrange("e (po pi) h -> e pi po h", pi=128)
        projT4 = projT.rearrange("e (po pi) c -> e pi po c", pi=128)
        with ExitStack() as s5:
            wpool = s5.enter_context(tc.tile_pool(name="s5w", bufs=4))
            pool = s5.enter_context(tc.tile_pool(name="s5", bufs=2))
            hps = s5.enter_context(tc.tile_pool(name="s5hp", bufs=2, space="PSUM"))
            ops = s5.enter_context(tc.tile_pool(name="s5op", bufs=2, space="PSUM"))
            for e in range(E_MLP):
                spr = pool.tile([1, NTOK], F32, tag="spr")
                nc.sync.dma_start(
                    spr[:], sparseT[e:e + 1, :, :].rearrange("p g n -> p (g n)"))
                wb_ps = hps.tile([128, NTOK], F32, tag="wbps")
                nc.tensor.matmul(wb_ps[:], ones_row[:], spr[:])
                wbc = pool.tile([128, NTOK], F32, tag="wbc")
                nc.vector.tensor_copy(wbc[:], wb_ps[:])
                h2w = pool.tile([128, CT, NTOK], F32R, tag="h2w")
                for m in range(8):
                    wch = wpool.tile([128, CT, 128], F32R, tag="wch")
                    nc.sync.dma_start(wch[:], fcT4[e, :, :, m * 128:(m + 1) * 128])
                    ph = hps.tile([128, NTOK], F32, tag="ph")
                    for ct in range(CT):
                        nc.tensor.matmul(ph[:], wch[:, ct, :], xfr[:, ct, :],
                                         start=(ct == 0), stop=(ct == CT - 1))
                    rl = pool.tile([128, NTOK], F32, tag="rl")
                    nc.vector.tensor_scalar_max(rl[:], ph[:], 0.0)
                    r2 = pool.tile([128, NTOK], F32, tag="r2")
                    nc.vector.tensor_mul(r2[:], rl[:], rl[:])
                    nc.vector.tensor_mul(h2w[:, m, :], r2[:], wbc[:])
                for ct in range(CT):
                    pch = wpool.tile([128, CT, 128], F32R, tag="wch")
                    nc.sync.dma_start(pch[:], projT4[e, :, :, ct * 128:(ct + 1) * 128])
                    po = ops.tile([128, NTOK], F32, tag="po")
                    for m in range(8):
                        nc.tensor.matmul(po[:], pch[:, m, :], h2w[:, m, :],
                                         start=(m == 0), stop=(m == 7))
                    nc.vector.tensor_add(moe_acc[:, ct, :], moe_acc[:, ct, :], po[:])

        # ---- VE experts + output ----
        with ExitStack() as s6:
            pool = s6.enter_context(tc.tile_pool(name="s6", bufs=1))
            ppool = s6.enter_context(tc.tile_pool(name="s6p", bufs=2, space="PSUM"))
            idx_sb = pool.tile([128, NTOK // 16], mybir.dt.int16, tag="idx")
            nc.sync.dma_start(idx_sb[:], tok_idx[:, :])
            g0 = pool.tile([128, 4, C], F32, tag="g0")
            g1 = pool.tile([128, 4, C], F32, tag="g1")
            nc.gpsimd.dma_gather(out_ap=g0[:], in_ap=ve[0], idxs_ap=idx_sb[:],
                                 num_idxs=NTOK, num_idxs_reg=NTOK, elem_size=C)
            nc.gpsimd.dma_gather(out_ap=g1[:], in_ap=ve[1], idxs_ap=idx_sb[:],
                                 num_idxs=NTOK, num_idxs_reg=NTOK, elem_size=C)
            vtok = pool.tile([128, 4, C], F32, tag="vtok")
            for g in range(4):
                nc.vector.tensor_scalar_mul(vtok[:, g, :], g0[:, g, :], ve_w[:, g, 0:1])
                nc.vector.scalar_tensor_tensor(vtok[:, g, :], g1[:, g, :], ve_w[:, g, 1:2],
                                               vtok[:, g, :], op0=OP.mult, op1=OP.add)
            for g in range(4):
                for ct in range(CT):
                    tp = ppool.tile([128, 128], F32, tag="tp")
                    nc.tensor.transpose(tp[:], vtok[:, g, ct * 128:(ct + 1) * 128], ident[:])
                    nc.vector.tensor_add(moe_acc[:, ct, g * 128:(g + 1) * 128],
                                         moe_acc[:, ct, g * 128:(g + 1) * 128], tp[:])
            outT3 = outT.rearrange("(po pi) n -> pi po n", pi=128)
            nc.sync.dma_start(outT3, moe_acc[:])

    nc.compile()
    return nc


_CACHE = {}


def _get_program(window_size, n_mask_slices):
    key = (int(window_size), int(n_mask_slices))
    if key not in _CACHE:
        _CACHE[key] = _build(*key)
    return _CACHE[key]


def make_in_maps(x, cos, sin, token_ids, window_size, wq, wk, wv, wo,
                 router_w, fc_w, proj_w, ve_tables):
    x = np.asarray(x, np.float32)
    w = int(np.asarray(window_size))
    _, mask_stack = _mask_plan(w)

    cosT = np.ascontiguousarray(np.asarray(cos, np.float32).reshape(T, 32).T)
    s32 = np.asarray(sin, np.float32).reshape(T, 32).T
    sinT = np.ascontiguousarray(np.concatenate([s32, -s32], axis=0))
    routerT = np.ascontiguousarray(np.asarray(router_w, np.float32).T)
    fcT = np.ascontiguousarray(np.asarray(fc_w, np.float32).transpose(0, 2, 1))
    projT = np.ascontiguousarray(np.asarray(proj_w, np.float32).transpose(0, 2, 1))
    ve = np.ascontiguousarray(np.asarray(ve_tables, np.float32))
    wqT_full = np.asarray(wq, np.float32).T
    wkT_full = np.asarray(wk, np.float32).T
    wvT_full = np.asarray(wv, np.float32).T
    woT_full = np.asarray(wo, np.float32).T
    token_ids = np.asarray(token_ids)

    ind_a = np.zeros((128, 2), np.float32)
    ind_a[0:64, 0] = 1.0
    ind_a[64:128, 1] = 1.0
    ind_b = np.ascontiguousarray(ind_a.T)

    in_maps = []
    for c in range(NCORES):
        b, g = c // 4, c % 4
        xT_b = np.ascontiguousarray(x[b].T)
        own = np.ascontiguousarray(xT_b[:, g * NTOK:(g + 1) * NTOK])
        tid = token_ids[b, g * NTOK:(g + 1) * NTOK].astype(np.int16)
        idx16 = np.zeros((16, NTOK // 16), np.int16)
        for i in range(NTOK):
            idx16[i % 16, i // 16] = tid[i]
        in_maps.append(dict(
            xT=xT_b,
            own_xT=own,
            cosT=cosT, sinT=sinT,
            wqT=np.ascontiguousarray(wqT_full[:, g * 256:(g + 1) * 256]),
            wkT=np.ascontiguousarray(wkT_full[:, g * 128:(g + 1) * 128]),
            wvT=np.ascontiguousarray(wvT_full[:, g * 128:(g + 1) * 128]),
            woT=np.ascontiguousarray(woT_full[g * 256:(g + 1) * 256, :]),
            routerT=routerT, fcT=fcT, projT=projT, ve=ve,
            tok_idx=np.tile(idx16, (8, 1)),
            masks=mask_stack,
            ind_a=ind_a, ind_b=ind_b,
        ))
    return in_maps, w, mask_stack.shape[0]


def assemble(results):
    out = np.zeros((B, T, C), np.float32)
    routing = np.zeros((B, T, E_TOT), np.float32)
    for c in range(NCORES):
        b, g = c // 4, c % 4
        sl = slice(g * NTOK, (g + 1) * NTOK)
        out[b, sl, :] = results[c]["outT"].T
        routing[b, sl, :] = results[c]["routT"].T
    return out, routing


def kernel(**inputs):
    in_maps, w, nslices = make_in_maps(**inputs)
    nc = _get_program(w, nslices)
    res = run_bass_kernel_spmd(nc, in_maps, core_ids=list(range(NCORES)))
    return assemble(res.results)
